# revision 8
# baseline (speedup 1.0000x reference)
"""Trainium2 Bass kernel for nn_DBFusion (gated dual-injection fusion + GroupNorm).

Reference computation (per batch sample b, C=64 channels, L=65536 positions):
    acc  = x * (gate_w @ (inj0 + x) + gate_b) + x * (gate_w @ (inj1 + x) + gate_b)
         = x * (gate_w @ (inj0 + inj1 + 2x) + 2*gate_b)          # affine fold
    out  = relu(fuse_w @ acc + fuse_b + residual)
    out  = GroupNorm(num_groups=1)(out) * gn_w + gn_b            # per-sample stats

Distribution: pure data parallel — batch dim B=8, one sample per NeuronCore.

Per-core layout: the [64, 65536] sample is folded to [128, 32768]: partitions
0:64 hold channels for L in [0, 32768), partitions 64:128 hold channels for
L in [32768, 65536). All matmuls use 128x128 block-diagonal weights so one
K=128 matmul processes both halves; all elementwise ops run at the full 128
partition width.

Matmuls run as float32r (TF32) — ~1.5e-4 relative error, full PE rate.
The inj0+inj1 sum is computed by the DMA engine (SWDGE accumulate-DMA), so
no compute engine pass is spent on it.
"""

import sys

if "/opt/trn_rl_repo" not in sys.path:
    sys.path.insert(0, "/opt/trn_rl_repo")

import numpy as np

B, C, L = 8, 64, 65536
H = L // 2  # 32768, per-half length
P = 128  # partitions
CB = 1024  # columns per DMA block (per half)
NB = H // CB  # 32 blocks
MM = 512  # matmul free-dim chunk (one PSUM bank)
SUB = CB // MM  # matmul sub-chunks per block
OB = 2048  # phase-2 output block columns
N_CORES = 8
GN_EPS = 1e-5

_cache = {}

# inj0+inj1 via SWDGE accumulate-DMA (True) or gpsimd tensor_add (False).
# The accumulate-DMA variant passes CoreSim but dies at runtime on HW
# (axon/NRT INTERNAL error), so the gpsimd add is the default.
USE_DMA_ACCUM = False


def _build_module():
    import concourse.mybir as mybir
    from concourse import bacc
    from concourse.tile import TileContext

    f32 = mybir.dt.float32
    f32r = mybir.dt.float32r
    ALU = mybir.AluOpType
    ACT = mybir.ActivationFunctionType

    nc = bacc.Bacc()

    x_d = nc.dram_tensor("x", [C, L], f32r, kind="ExternalInput")
    i0_d = nc.dram_tensor("inj0", [C, L], f32r, kind="ExternalInput")
    i1_d = nc.dram_tensor("inj1", [C, L], f32r, kind="ExternalInput")
    rs_d = nc.dram_tensor("res", [C, L], f32r, kind="ExternalInput")
    # wts columns: [0:128]=blockdiag(gw.T), [128:256]=blockdiag(2gw.T),
    #              [256:384]=blockdiag(fw.T), [384:512]=I_128
    w_d = nc.dram_tensor("wts", [P, 4 * P], f32r, kind="ExternalInput")
    # params columns: 0=2*gate_b, 1=fuse_b, 2=gn_w, 3=gn_b (each tiled x2)
    p_d = nc.dram_tensor("params", [P, 4], f32, kind="ExternalInput")
    o_d = nc.dram_tensor("out", [P, H], f32, kind="ExternalOutput")

    # fold [C, L] -> [half, C, H]; DMA'd to [128, cb] tiles with partition
    # p = half*64 + c (the 3D DRAM pattern maps elementwise onto the tile)
    xr = x_d[:, :].rearrange("c (h l) -> h c l", h=2)
    i0r = i0_d[:, :].rearrange("c (h l) -> h c l", h=2)
    i1r = i1_d[:, :].rearrange("c (h l) -> h c l", h=2)
    rsr = rs_d[:, :].rearrange("c (h l) -> h c l", h=2)

    with TileContext(nc) as tc:
        with (
            tc.tile_pool(name="singles", bufs=1) as singles,
            tc.tile_pool(name="work", bufs=2) as work,
            tc.tile_pool(name="psum", bufs=2, space="PSUM") as psum,
            tc.tile_pool(name="psum1", bufs=1, space="PSUM") as psum1,
        ):
            wts = singles.tile([P, 4 * P], f32r)
            nc.sync.dma_start(wts, w_d[:, :])
            params = singles.tile([P, 4], f32)
            nc.sync.dma_start(params, p_d[:, :])

            resident = singles.tile([P, H], f32)
            stats = singles.tile([P, 2 * NB, 6], f32)

            w_gt = wts[:, 0:128]
            w_gx = wts[:, 128:256]
            w_f = wts[:, 256:384]
            w_i = wts[:, 384:512]
            gb2 = params[:, 0:1]
            fb = params[:, 1:2]

            for j in range(NB):
                cols = slice(j * CB, (j + 1) * CB)
                # T = inj0 + inj1, computed by the DMA engine (accumulate-DMA)
                T = work.tile([P, CB], f32r, tag="T")
                if USE_DMA_ACCUM:
                    nc.gpsimd.dma_start(T[:, :], i0r[:, :, cols])
                    nc.gpsimd.dma_start(T[:, :], i1r[:, :, cols], accum_op=ALU.add)
                else:
                    J0 = work.tile([P, CB], f32r, tag="J0")
                    nc.sync.dma_start(J0[:, :], i0r[:, :, cols])
                    J1 = work.tile([P, CB], f32r, tag="J1")
                    nc.sync.dma_start(J1[:, :], i1r[:, :, cols])
                    J0 = J0[:, :].bitcast(f32)
                    J1 = J1[:, :].bitcast(f32)
                    nc.gpsimd.tensor_add(T[:, :], J0[:, :], J1[:, :])
                X = work.tile([P, CB], f32r, tag="X")
                nc.sync.dma_start(X[:, :], xr[:, :, cols])
                R = work.tile([P, CB], f32r, tag="R")
                nc.sync.dma_start(R[:, :], rsr[:, :, cols])

                for s in range(SUB):
                    sl = slice(s * MM, (s + 1) * MM)
                    c0 = j * CB + s * MM
                    # psum_g = gw @ (inj0+inj1) + 2gw @ x   (both halves at once)
                    pg = psum.tile([P, MM], f32, tag="pg")
                    nc.tensor.matmul(pg, w_gt, T[:, sl], start=True, stop=False)
                    nc.tensor.matmul(pg, w_gx, X[:, sl], start=False, stop=True)
                    # acc = (psum_g + 2*gate_b) * x
                    ACCT = work.tile([P, MM], f32r, tag="ACCT")
                    nc.vector.scalar_tensor_tensor(
                        out=ACCT[:, :],
                        in0=pg[:, :],
                        scalar=gb2,
                        in1=X[:, sl].bitcast(f32),
                        op0=ALU.add,
                        op1=ALU.mult,
                    )
                    # psum_f = fw @ acc + I @ residual
                    pf = psum.tile([P, MM], f32, tag="pf")
                    nc.tensor.matmul(pf, w_f, ACCT[:, :], start=True, stop=False)
                    nc.tensor.matmul(pf, w_i, R[:, sl], start=False, stop=True)
                    # resident = relu(psum_f + fuse_b)
                    nc.scalar.activation(
                        out=resident[:, c0 : c0 + MM],
                        in_=pf[:, :],
                        func=ACT.Relu,
                        bias=fb,
                        scale=1.0,
                    )
                    nc.vector.bn_stats(
                        out=stats[:, 2 * j + s, :],
                        in_=resident[:, c0 : c0 + MM],
                    )

            # ---- per-sample GroupNorm statistics ----
            mv = singles.tile([P, 2], f32)
            nc.vector.bn_aggr(out=mv, in_=stats[:, :, :])

            # ST = [mean_p, E[x^2]_p]
            ST = singles.tile([P, 2], f32)
            nc.gpsimd.tensor_copy(out=ST[:, 0:1], in_=mv[:, 0:1])
            nc.vector.scalar_tensor_tensor(
                out=ST[:, 1:2],
                in0=mv[:, 0:1],
                scalar=mv[:, 0:1],
                in1=mv[:, 1:2],
                op0=ALU.mult,
                op1=ALU.add,
            )
            # cross-partition reduce: pr[0, :] = sum_p ST[p, :]  (full-fp32 matmul)
            ones_col = singles.tile([P, 1], f32)
            nc.vector.memset(ones_col, 1.0)
            pr = psum1.tile([1, 2], f32, tag="pr")
            nc.tensor.matmul(pr, ones_col[:, :], ST[:, :], start=True, stop=True)
            vt = singles.tile([1, 2], f32)
            nc.scalar.copy(vt, pr[:, :])
            # broadcast back to 128 partitions: pb = ones_row.T @ vt
            ones_row = singles.tile([1, P], f32)
            nc.vector.memset(ones_row, 1.0)
            pb = psum1.tile([P, 2], f32, tag="pb")
            nc.tensor.matmul(pb, ones_row[:, :], vt[:, :], start=True, stop=True)

            G = singles.tile([P, 8], f32)
            mean = G[:, 0:1]
            ex2 = G[:, 1:2]
            negvar = G[:, 2:3]
            sd = G[:, 3:4]
            rstd = G[:, 4:5]
            A = G[:, 5:6]
            negma = G[:, 6:7]
            Bb = G[:, 7:8]
            nc.scalar.mul(mean, pb[:, 0:1], 1.0 / P)
            nc.scalar.mul(ex2, pb[:, 1:2], 1.0 / P)
            # negvar = mean^2 - E[x^2]
            nc.vector.scalar_tensor_tensor(
                out=negvar,
                in0=mean,
                scalar=mean,
                in1=ex2,
                op0=ALU.mult,
                op1=ALU.subtract,
            )
            # sd = sqrt(var + eps) ; rstd = 1/sd
            eps_t = singles.tile([P, 1], f32)
            nc.vector.memset(eps_t, GN_EPS)
            nc.scalar.activation(
                out=sd, in_=negvar, func=ACT.Sqrt, bias=eps_t, scale=-1.0
            )
            nc.vector.reciprocal(out=rstd, in_=sd)
            nc.vector.tensor_mul(A, rstd, params[:, 2:3])
            nc.vector.tensor_scalar(
                out=negma,
                in0=mean,
                scalar1=A,
                scalar2=-1.0,
                op0=ALU.mult,
                op1=ALU.mult,
            )
            nc.vector.tensor_add(Bb, negma, params[:, 3:4])

            # ---- phase 2: out = resident * A + B ----
            for j2 in range(H // OB):
                cols = slice(j2 * OB, (j2 + 1) * OB)
                bounce = work.tile([P, OB], f32, tag="bounce")
                nc.scalar.activation(
                    out=bounce[:, :],
                    in_=resident[:, cols],
                    func=ACT.Identity,
                    bias=Bb,
                    scale=A,
                )
                nc.sync.dma_start(o_d[:, cols], bounce[:, :])

    nc.finalize()
    return nc


def _prep_shared(gate_w, gate_b, fuse_w, fuse_b, gn_w, gn_b):
    gwT = gate_w.T.astype(np.float32)
    fwT = fuse_w.T.astype(np.float32)
    wts = np.zeros((P, 4 * P), dtype=np.float32)
    wts[0:64, 0:64] = gwT
    wts[64:128, 64:128] = gwT
    wts[0:64, 128:192] = 2.0 * gwT
    wts[64:128, 192:256] = 2.0 * gwT
    wts[0:64, 256:320] = fwT
    wts[64:128, 320:384] = fwT
    wts[:, 384:512] = np.eye(P, dtype=np.float32)

    params = np.zeros((P, 4), dtype=np.float32)
    params[:, 0] = np.tile(2.0 * gate_b, 2)
    params[:, 1] = np.tile(fuse_b, 2)
    params[:, 2] = np.tile(gn_w, 2)
    params[:, 3] = np.tile(gn_b, 2)
    return wts, params


def kernel(
    x, inj0, inj1, residual, gate_w, gate_b, fuse_w, fuse_b, gn_w, gn_b, trace=False
):
    from concourse.bass_utils import run_bass_kernel_spmd

    x = np.ascontiguousarray(np.asarray(x, dtype=np.float32))
    inj0 = np.ascontiguousarray(np.asarray(inj0, dtype=np.float32))
    inj1 = np.ascontiguousarray(np.asarray(inj1, dtype=np.float32))
    residual = np.ascontiguousarray(np.asarray(residual, dtype=np.float32))
    gate_w = np.asarray(gate_w, dtype=np.float32)
    gate_b = np.asarray(gate_b, dtype=np.float32)
    fuse_w = np.asarray(fuse_w, dtype=np.float32)
    fuse_b = np.asarray(fuse_b, dtype=np.float32)
    gn_w = np.asarray(gn_w, dtype=np.float32)
    gn_b = np.asarray(gn_b, dtype=np.float32)

    if "nc" not in _cache:
        _cache["nc"] = _build_module()
    nc = _cache["nc"]

    wts, params = _prep_shared(gate_w, gate_b, fuse_w, fuse_b, gn_w, gn_b)

    in_maps = []
    for b in range(N_CORES):
        in_maps.append(
            {
                "x": x[b],
                "inj0": inj0[b],
                "inj1": inj1[b],
                "res": residual[b],
                "wts": wts,
                "params": params,
            }
        )

    res = run_bass_kernel_spmd(
        nc, in_maps, core_ids=list(range(N_CORES)), trace=trace
    )

    out = np.empty((B, C, L), dtype=np.float32)
    for b in range(N_CORES):
        o = res.results[b]["out"]  # [128, 32768]
        out[b, :, :H] = o[0:64]
        out[b, :, H:] = o[64:128]
    if trace:
        _cache["last_result"] = res
    return out


# revision 9
# speedup vs baseline: 4.6453x; 4.6453x over previous
"""Trainium2 Bass kernel for nn_DBFusion (gated dual-injection fusion + GroupNorm).

Reference computation (per batch sample b, C=64 channels, L=65536 positions):
    acc  = x * (gate_w @ (inj0 + x) + gate_b) + x * (gate_w @ (inj1 + x) + gate_b)
         = x * (gate_w @ (inj0 + inj1 + 2x) + 2*gate_b)          # affine fold
    out  = relu(fuse_w @ acc + fuse_b + residual)
    out  = GroupNorm(num_groups=1)(out) * gn_w + gn_b            # per-sample stats

Distribution: pure data parallel — batch dim B=8, one sample per NeuronCore.

Per-core layout: the [64, 65536] sample is folded to [128, 32768]: partitions
0:64 hold channels for L in [0, 32768), partitions 64:128 hold channels for
L in [32768, 65536). All matmuls use 128x128 block-diagonal weights so one
K=128 matmul processes both halves; all elementwise ops run at the full 128
partition width.

Matmuls run as float32r (TF32) — ~1.5e-4 relative error, full PE rate.
The inj0+inj1 sum is computed by the DMA engine (SWDGE accumulate-DMA), so
no compute engine pass is spent on it.
"""

import sys

if "/opt/trn_rl_repo" not in sys.path:
    sys.path.insert(0, "/opt/trn_rl_repo")

import numpy as np

B, C, L = 8, 64, 65536
H = L // 2  # 32768, per-half length
P = 128  # partitions
CB = 1024  # columns per DMA block (per half)
NB = H // CB  # 32 blocks
MM = 512  # matmul free-dim chunk (one PSUM bank)
SUB = CB // MM  # matmul sub-chunks per block
OB = 2048  # phase-2 output block columns
N_CORES = 8
GN_EPS = 1e-5

_cache = {}

# inj0+inj1 via SWDGE accumulate-DMA (True) or gpsimd tensor_add (False).
# The accumulate-DMA variant passes CoreSim but dies at runtime on HW
# (axon/NRT INTERNAL error), so the gpsimd add is the default.
USE_DMA_ACCUM = False


def _build_module():
    import concourse.mybir as mybir
    from concourse import bacc
    from concourse.tile import TileContext

    f32 = mybir.dt.float32
    f32r = mybir.dt.float32r
    ALU = mybir.AluOpType
    ACT = mybir.ActivationFunctionType

    nc = bacc.Bacc()

    x_d = nc.dram_tensor("x", [C, L], f32r, kind="ExternalInput")
    i0_d = nc.dram_tensor("inj0", [C, L], f32r, kind="ExternalInput")
    i1_d = nc.dram_tensor("inj1", [C, L], f32r, kind="ExternalInput")
    rs_d = nc.dram_tensor("res", [C, L], f32r, kind="ExternalInput")
    # wts columns: [0:128]=blockdiag(gw.T), [128:256]=blockdiag(2gw.T),
    #              [256:384]=blockdiag(fw.T), [384:512]=I_128
    w_d = nc.dram_tensor("wts", [P, 4 * P], f32r, kind="ExternalInput")
    # params columns: 0=2*gate_b, 1=fuse_b, 2=gn_w, 3=gn_b (each tiled x2)
    p_d = nc.dram_tensor("params", [P, 4], f32, kind="ExternalInput")
    o_d = nc.dram_tensor("out", [P, H], f32, kind="ExternalOutput")

    # fold [C, L] -> [C, half, H]; DMA'd to [128, cb] tiles with partition
    # p = c*2 + half. The outermost DRAM AP dim is 64 (not 2) so each DMA
    # fans out across all 16 SDMA engines (outer-dim-2 patterns only got 2
    # engines / ~52 GB/s on HW).
    xr = x_d[:, :].rearrange("c (h l) -> c h l", h=2)
    i0r = i0_d[:, :].rearrange("c (h l) -> c h l", h=2)
    i1r = i1_d[:, :].rearrange("c (h l) -> c h l", h=2)
    rsr = rs_d[:, :].rearrange("c (h l) -> c h l", h=2)

    with TileContext(nc) as tc:
        with (
            tc.tile_pool(name="singles", bufs=1) as singles,
            tc.tile_pool(name="work", bufs=2) as work,
            tc.tile_pool(name="psum", bufs=2, space="PSUM") as psum,
            tc.tile_pool(name="psum1", bufs=1, space="PSUM") as psum1,
        ):
            wts = singles.tile([P, 4 * P], f32r)
            nc.sync.dma_start(wts, w_d[:, :])
            params = singles.tile([P, 4], f32)
            nc.sync.dma_start(params, p_d[:, :])

            resident = singles.tile([P, H], f32)
            stats = singles.tile([P, 2 * NB, 6], f32)

            w_gt = wts[:, 0:128]
            w_gx = wts[:, 128:256]
            w_f = wts[:, 256:384]
            w_i = wts[:, 384:512]
            gb2 = params[:, 0:1]
            fb = params[:, 1:2]

            for j in range(NB):
                cols = slice(j * CB, (j + 1) * CB)
                # T = inj0 + inj1, computed by the DMA engine (accumulate-DMA)
                T = work.tile([P, CB], f32r, tag="T")
                if USE_DMA_ACCUM:
                    nc.gpsimd.dma_start(T[:, :], i0r[:, :, cols])
                    nc.gpsimd.dma_start(T[:, :], i1r[:, :, cols], accum_op=ALU.add)
                else:
                    J0 = work.tile([P, CB], f32r, tag="J0")
                    nc.sync.dma_start(J0[:, :], i0r[:, :, cols])
                    J1 = work.tile([P, CB], f32r, tag="J1")
                    nc.scalar.dma_start(J1[:, :], i1r[:, :, cols])
                    J0 = J0[:, :].bitcast(f32)
                    J1 = J1[:, :].bitcast(f32)
                    nc.gpsimd.tensor_add(T[:, :], J0[:, :], J1[:, :])
                X = work.tile([P, CB], f32r, tag="X")
                nc.sync.dma_start(X[:, :], xr[:, :, cols])
                R = work.tile([P, CB], f32r, tag="R")
                nc.scalar.dma_start(R[:, :], rsr[:, :, cols])

                for s in range(SUB):
                    sl = slice(s * MM, (s + 1) * MM)
                    c0 = j * CB + s * MM
                    # psum_g = gw @ (inj0+inj1) + 2gw @ x   (both halves at once)
                    pg = psum.tile([P, MM], f32, tag="pg")
                    nc.tensor.matmul(pg, w_gt, T[:, sl], start=True, stop=False)
                    nc.tensor.matmul(pg, w_gx, X[:, sl], start=False, stop=True)
                    # acc = (psum_g + 2*gate_b) * x
                    ACCT = work.tile([P, MM], f32r, tag="ACCT")
                    nc.vector.scalar_tensor_tensor(
                        out=ACCT[:, :],
                        in0=pg[:, :],
                        scalar=gb2,
                        in1=X[:, sl].bitcast(f32),
                        op0=ALU.add,
                        op1=ALU.mult,
                    )
                    # psum_f = fw @ acc + I @ residual
                    pf = psum.tile([P, MM], f32, tag="pf")
                    nc.tensor.matmul(pf, w_f, ACCT[:, :], start=True, stop=False)
                    nc.tensor.matmul(pf, w_i, R[:, sl], start=False, stop=True)
                    # resident = relu(psum_f + fuse_b)
                    nc.scalar.activation(
                        out=resident[:, c0 : c0 + MM],
                        in_=pf[:, :],
                        func=ACT.Relu,
                        bias=fb,
                        scale=1.0,
                    )
                    nc.vector.bn_stats(
                        out=stats[:, 2 * j + s, :],
                        in_=resident[:, c0 : c0 + MM],
                    )

            # ---- per-sample GroupNorm statistics ----
            mv = singles.tile([P, 2], f32)
            nc.vector.bn_aggr(out=mv, in_=stats[:, :, :])

            # ST = [mean_p, E[x^2]_p]
            ST = singles.tile([P, 2], f32)
            nc.gpsimd.tensor_copy(out=ST[:, 0:1], in_=mv[:, 0:1])
            nc.vector.scalar_tensor_tensor(
                out=ST[:, 1:2],
                in0=mv[:, 0:1],
                scalar=mv[:, 0:1],
                in1=mv[:, 1:2],
                op0=ALU.mult,
                op1=ALU.add,
            )
            # cross-partition reduce: pr[0, :] = sum_p ST[p, :]  (full-fp32 matmul)
            ones_col = singles.tile([P, 1], f32)
            nc.vector.memset(ones_col, 1.0)
            pr = psum1.tile([1, 2], f32, tag="pr")
            nc.tensor.matmul(pr, ones_col[:, :], ST[:, :], start=True, stop=True)
            vt = singles.tile([1, 2], f32)
            nc.scalar.copy(vt, pr[:, :])
            # broadcast back to 128 partitions: pb = ones_row.T @ vt
            ones_row = singles.tile([1, P], f32)
            nc.vector.memset(ones_row, 1.0)
            pb = psum1.tile([P, 2], f32, tag="pb")
            nc.tensor.matmul(pb, ones_row[:, :], vt[:, :], start=True, stop=True)

            G = singles.tile([P, 8], f32)
            mean = G[:, 0:1]
            ex2 = G[:, 1:2]
            negvar = G[:, 2:3]
            sd = G[:, 3:4]
            rstd = G[:, 4:5]
            A = G[:, 5:6]
            negma = G[:, 6:7]
            Bb = G[:, 7:8]
            nc.scalar.mul(mean, pb[:, 0:1], 1.0 / P)
            nc.scalar.mul(ex2, pb[:, 1:2], 1.0 / P)
            # negvar = mean^2 - E[x^2]
            nc.vector.scalar_tensor_tensor(
                out=negvar,
                in0=mean,
                scalar=mean,
                in1=ex2,
                op0=ALU.mult,
                op1=ALU.subtract,
            )
            # sd = sqrt(var + eps) ; rstd = 1/sd
            eps_t = singles.tile([P, 1], f32)
            nc.vector.memset(eps_t, GN_EPS)
            nc.scalar.activation(
                out=sd, in_=negvar, func=ACT.Sqrt, bias=eps_t, scale=-1.0
            )
            nc.vector.reciprocal(out=rstd, in_=sd)
            nc.vector.tensor_mul(A, rstd, params[:, 2:3])
            nc.vector.tensor_scalar(
                out=negma,
                in0=mean,
                scalar1=A,
                scalar2=-1.0,
                op0=ALU.mult,
                op1=ALU.mult,
            )
            nc.vector.tensor_add(Bb, negma, params[:, 3:4])

            # ---- phase 2: out = resident * A + B ----
            for j2 in range(H // OB):
                cols = slice(j2 * OB, (j2 + 1) * OB)
                bounce = work.tile([P, OB], f32, tag="bounce")
                nc.scalar.activation(
                    out=bounce[:, :],
                    in_=resident[:, cols],
                    func=ACT.Identity,
                    bias=Bb,
                    scale=A,
                )
                nc.sync.dma_start(o_d[:, cols], bounce[:, :])

    nc.finalize()
    return nc


def _prep_shared(gate_w, gate_b, fuse_w, fuse_b, gn_w, gn_b):
    # partition p = 2*c + half  ->  weights are kron(w.T, I2)
    i2 = np.eye(2, dtype=np.float32)
    gwT = gate_w.T.astype(np.float32)
    fwT = fuse_w.T.astype(np.float32)
    wts = np.zeros((P, 4 * P), dtype=np.float32)
    wts[:, 0:128] = np.kron(gwT, i2)
    wts[:, 128:256] = np.kron(2.0 * gwT, i2)
    wts[:, 256:384] = np.kron(fwT, i2)
    wts[:, 384:512] = np.eye(P, dtype=np.float32)

    params = np.zeros((P, 4), dtype=np.float32)
    params[:, 0] = np.repeat(2.0 * gate_b, 2)
    params[:, 1] = np.repeat(fuse_b, 2)
    params[:, 2] = np.repeat(gn_w, 2)
    params[:, 3] = np.repeat(gn_b, 2)
    return wts, params


def kernel(
    x, inj0, inj1, residual, gate_w, gate_b, fuse_w, fuse_b, gn_w, gn_b, trace=False
):
    from concourse.bass_utils import run_bass_kernel_spmd

    x = np.ascontiguousarray(np.asarray(x, dtype=np.float32))
    inj0 = np.ascontiguousarray(np.asarray(inj0, dtype=np.float32))
    inj1 = np.ascontiguousarray(np.asarray(inj1, dtype=np.float32))
    residual = np.ascontiguousarray(np.asarray(residual, dtype=np.float32))
    gate_w = np.asarray(gate_w, dtype=np.float32)
    gate_b = np.asarray(gate_b, dtype=np.float32)
    fuse_w = np.asarray(fuse_w, dtype=np.float32)
    fuse_b = np.asarray(fuse_b, dtype=np.float32)
    gn_w = np.asarray(gn_w, dtype=np.float32)
    gn_b = np.asarray(gn_b, dtype=np.float32)

    if "nc" not in _cache:
        _cache["nc"] = _build_module()
    nc = _cache["nc"]

    wts, params = _prep_shared(gate_w, gate_b, fuse_w, fuse_b, gn_w, gn_b)

    in_maps = []
    for b in range(N_CORES):
        in_maps.append(
            {
                "x": x[b],
                "inj0": inj0[b],
                "inj1": inj1[b],
                "res": residual[b],
                "wts": wts,
                "params": params,
            }
        )

    res = run_bass_kernel_spmd(
        nc, in_maps, core_ids=list(range(N_CORES)), trace=trace
    )

    out = np.empty((B, C, L), dtype=np.float32)
    for b in range(N_CORES):
        o = res.results[b]["out"]  # [128, 32768], partition p = 2*c + half
        out[b] = o.reshape(C, L)
    if trace:
        _cache["last_result"] = res
    return out
